# revision 5
# baseline (speedup 1.0000x reference)
"""Distributed Trainium2 attention-block kernel (8 NeuronCores).

Problem: y = LN(x) -> QKV -> 16-head attention (seq 2048, dh 64) -> out-proj.
x [2,2048,1024] f32.

Sharding: token-parallel. Core c handles batch c//4, token quarter c%4
(512 query tokens). Each core computes Q,K,V for its own 512 tokens
(all heads), AllGathers K^T and V within its 4-core batch group, then
runs attention for its 512 queries over the full 2048-token sequence
and the final projection. Output shards are disjoint -> no reduction.

All matmuls run in float32r (tf32-like: full bf16-rate on TensorE for
free-dim >= 256, ~1.5e-4 matmul rel err measured on HW). f32r tiles are
DMA'd straight from f32 DRAM via bitcast - the PE rounds on read, so no
cast passes are needed anywhere.

Attention per head: dots computed transposed (k on partitions, q free)
so softmax-exp'd probabilities feed PV directly as the moving operand;
PV's stationary is [V_tile | ones] (M=65) so the softmax denominator
accumulates in PSUM row 64 for free. exp (ScalarE) reads dots PSUM in
batches of 3 k-tiles to amortize ACTIVATE instruction overhead. No
max-subtraction: scaled dots are ~N(0,1) (LN'd x, w_qkv ~ N(0,1/d)),
max over all scores ~6 => exp <= ~500, safe in f32.
"""

import os
import numpy as np

import concourse.bass as bass
import concourse.tile as tile
from concourse import mybir
from concourse.bass_utils import run_bass_kernel_spmd
from concourse.masks import make_identity

F32 = mybir.dt.float32
F32R = mybir.dt.float32r

B, S, D = 2, 2048, 1024
H, DH = 16, 64
T = 512           # query tokens per core
P = 128
NKT = S // P      # 16 k-tiles
LN_EPS = 1e-5
SCALE = DH ** -0.5
EXP_BATCH = 3     # k-tiles per exp ACTIVATE call

_MAXW = 1


def _split_multiwaits(nc):
    """This container's walrus rejects >1 sync wait/update per instruction.
    Move extras onto adjacent same-engine NoOps."""
    import bass_rust

    for bb in nc.main_func.blocks:
        new_insts = []
        for inst in bb.instructions:
            si = inst.sync_info
            pre, post = [], []
            if si is not None:
                waits = list(si.on_wait or [])
                ups = list(si.on_update or [])
                if len(waits) > _MAXW or len(ups) > _MAXW:
                    for i in range(_MAXW, len(waits), _MAXW):
                        pre.append(bass_rust.InstNoOp(
                            name=f"I-{nc.next_id()}", engine=inst.engine,
                            ins=[], outs=[],
                            sync_info=mybir.SyncInfo(
                                on_wait=waits[i:i + _MAXW], on_update=[])))
                    for i in range(_MAXW, len(ups), _MAXW):
                        post.append(bass_rust.InstNoOp(
                            name=f"I-{nc.next_id()}", engine=inst.engine,
                            ins=[], outs=[],
                            sync_info=mybir.SyncInfo(
                                on_wait=[], on_update=ups[i:i + _MAXW])))
                    inst.sync_info = mybir.SyncInfo(
                        on_wait=waits[:_MAXW], on_update=ups[:_MAXW])
            new_insts.extend(pre)
            new_insts.append(inst)
            new_insts.extend(post)
        bb.instructions[:] = new_insts


def _maybe_install_ntff_hook():
    """Optional NTFF profiling support (BASS_TRACE=1); harmless if absent."""
    if not os.environ.get("BASS_TRACE"):
        return
    import sys
    import types
    if "antenv.axon_hooks" in sys.modules:
        return
    try:
        mod = types.ModuleType("antenv.axon_hooks")
        _h = [None]
        mod.set_axon_ntff_profile_hook = lambda h: _h.__setitem__(0, h)
        mod.get_axon_ntff_profile_hook = lambda: _h[0]
        import antenv
        from trn_agent_boot.trn_boot import _ntff_profile_via_ctypes
        hook = _ntff_profile_via_ctypes('/opt/axon/libaxon_pjrt.so')
        sys.modules["antenv.axon_hooks"] = mod
        antenv.axon_hooks = mod
        mod.set_axon_ntff_profile_hook(hook)
    except Exception:
        pass


def build(apply_ln_affine, apply_b_out):
    nc = bass.Bass()

    x_ext = nc.declare_dram_parameter("x", [T, D], F32, isOutput=False)
    gamma_ext = nc.declare_dram_parameter("ln_gamma", [1, D], F32, isOutput=False)
    beta_ext = nc.declare_dram_parameter("ln_beta", [1, D], F32, isOutput=False)
    wqkv_ext = nc.declare_dram_parameter("w_qkv", [D, 3 * D], F32, isOutput=False)
    wout_ext = nc.declare_dram_parameter("w_out", [D, D], F32, isOutput=False)
    bout_ext = nc.declare_dram_parameter("b_out", [1, D], F32, isOutput=False)
    out_ext = nc.declare_dram_parameter("out", [T, D], F32, isOutput=True)

    groups = [[0, 1, 2, 3], [4, 5, 6, 7]]
    NDT = D // P   # 8 contraction tiles over model dim
    NTT = T // P   # 4 token tiles per core
    NHP = H // 2   # 8 head pairs

    from contextlib import ExitStack
    with tile.TileContext(nc) as tc, ExitStack() as stack:
        consts = stack.enter_context(tc.tile_pool(name="consts", bufs=1))
        sb_main = stack.enter_context(tc.tile_pool(name="sb_main", bufs=1))
        dram = stack.enter_context(tc.tile_pool(name="dram", bufs=1, space="DRAM"))

        ident = consts.tile([P, P], F32)
        make_identity(nc, ident)
        eps_t = consts.tile([P, 1], F32)
        nc.vector.memset(eps_t, LN_EPS)
        ones2f = consts.tile([P, 2], F32)
        nc.vector.memset(ones2f, 1.0)

        if apply_ln_affine:
            gammaB = consts.tile([P, D], F32)
            betaB = consts.tile([P, D], F32)
            nc.sync.dma_start(out=gammaB, in_=bass.AP(
                tensor=gamma_ext.tensor, offset=gamma_ext.offset,
                ap=[[0, P]] + gamma_ext.ap[1:]))
            nc.sync.dma_start(out=betaB, in_=bass.AP(
                tensor=beta_ext.tensor, offset=beta_ext.offset,
                ap=[[0, P]] + beta_ext.ap[1:]))
        if apply_b_out:
            boutB = consts.tile([P, D], F32)
            nc.sync.dma_start(out=boutB, in_=bass.AP(
                tensor=bout_ext.tensor, offset=bout_ext.offset,
                ap=[[0, P]] + bout_ext.ap[1:]))

        # persistent activations
        xnT = [sb_main.tile([P, T], F32R, tag=f"xnT{i}", name=f"xnT{i}") for i in range(NDT)]
        qT = [sb_main.tile([P, T], F32R, tag=f"qT{i}", name=f"qT{i}") for i in range(NHP)]
        attnT = [sb_main.tile([P, T], F32R, tag=f"attnT{i}", name=f"attnT{i}") for i in range(NHP)]
        wout_sb = [sb_main.tile([P, D], F32R, tag=f"wout{i}", name=f"wout{i}") for i in range(NDT)]

        # AG buffers (internal DRAM)
        k_in = dram.tile([D, T], F32)
        v_in = dram.tile([T, D], F32)
        k_out = dram.tile([4 * D, T], F32)
        v_out = dram.tile([S, D], F32)
        recip_d = dram.tile([H, T], F32)

        # ---------------- Phase 1: LayerNorm + transpose ----------------
        with tc.tile_pool(name="p1sb", bufs=3) as p1sb, \
             tc.tile_pool(name="p1ps", bufs=4, space="PSUM") as p1ps:
            for tt in range(NTT):
                x_t = p1sb.tile([P, D], F32, tag="x")
                nc.sync.dma_start(out=x_t, in_=x_ext[tt * P:(tt + 1) * P, :])
                stats = p1sb.tile([P, 2, nc.vector.BN_STATS_DIM], F32, tag="st")
                for sg in range(2):
                    nc.vector.bn_stats(out=stats[:, sg, :],
                                       in_=x_t[:, sg * 512:(sg + 1) * 512])
                mv = p1sb.tile([P, nc.vector.BN_AGGR_DIM], F32, tag="mv")
                nc.vector.bn_aggr(out=mv, in_=stats)
                rstd = p1sb.tile([P, 1], F32, tag="rstd")
                nc.scalar.activation(out=rstd, in_=mv[:, 1:2],
                                     func=mybir.ActivationFunctionType.Sqrt,
                                     bias=eps_t, scale=1.0)
                nc.vector.reciprocal(out=rstd, in_=rstd)
                xn_t = p1sb.tile([P, D], F32, tag="xn")
                nc.vector.tensor_scalar(
                    out=xn_t, in0=x_t, scalar1=mv[:, 0:1], scalar2=rstd,
                    op0=mybir.AluOpType.subtract, op1=mybir.AluOpType.mult)
                if apply_ln_affine:
                    nc.vector.tensor_mul(out=xn_t, in0=xn_t, in1=gammaB)
                    nc.vector.tensor_add(out=xn_t, in0=xn_t, in1=betaB)
                for dt in range(NDT):
                    ps_tr = p1ps.tile([P, P], F32, tag="tr")
                    nc.tensor.transpose(ps_tr, xn_t[:, dt * P:(dt + 1) * P], ident)
                    nc.vector.tensor_copy(out=xnT[dt][:, tt * P:(tt + 1) * P],
                                          in_=ps_tr)

        # ---------------- Phase 2: QKV projection + AllGather ----------------
        # feature chunks: 0..7 q head-pairs, 8..15 k head-pairs, then v.
        with tc.tile_pool(name="p2sb", bufs=3) as p2sb, \
             tc.tile_pool(name="p2k", bufs=3) as p2k, \
             tc.tile_pool(name="p2ps", bufs=3, space="PSUM") as p2ps:
            wq_view = wqkv_ext.rearrange("(dt p) f -> dt p f", p=P)

            def proj_qk(ct, dst):
                w_c = p2sb.tile([P, NDT, P], F32R, tag="wqk")
                nc.sync.dma_start(
                    out=w_c,
                    in_=wq_view[:, :, ct * P:(ct + 1) * P]
                    .rearrange("dt p f -> p dt f").bitcast(F32R))
                ps = p2ps.tile([P, T], F32, tag="pqk")
                for dt in range(NDT):
                    nc.tensor.matmul(ps, w_c[:, dt, :], xnT[dt],
                                     start=(dt == 0), stop=(dt == NDT - 1))
                nc.vector.tensor_copy(out=dst, in_=ps)

            # k first (feeds AG), then v, then q (overlaps AG)
            kT_local = [p2k.tile([P, T], F32R, tag=f"kT{i}", name=f"kT{i}") for i in range(NHP)]
            for ct in range(NHP):
                proj_qk(NHP + ct, kT_local[ct])
                nc.sync.dma_start(
                    out=k_in[ct * P:(ct + 1) * P, :],
                    in_=kT_local[ct].bitcast(F32))
            nc.gpsimd.collective_compute(
                "AllGather", mybir.AluOpType.bypass, replica_groups=groups,
                ins=[k_in.opt()], outs=[k_out.opt()])

            for vc in range(2):
                wv_c = p2sb.tile([P, NDT, T], F32R, tag="wv")
                nc.sync.dma_start(
                    out=wv_c,
                    in_=wq_view[:, :, 2 * D + vc * T: 2 * D + (vc + 1) * T]
                    .rearrange("dt p f -> p dt f").bitcast(F32R))
                for vt_i in range(NTT):
                    ps = p2ps.tile([P, T], F32, tag="pv")
                    for dt in range(NDT):
                        nc.tensor.matmul(
                            ps, xnT[dt][:, vt_i * P:(vt_i + 1) * P],
                            wv_c[:, dt, :],
                            start=(dt == 0), stop=(dt == NDT - 1))
                    v_l = p2sb.tile([P, T], F32, tag="vl")
                    nc.vector.tensor_copy(out=v_l, in_=ps)
                    nc.sync.dma_start(
                        out=v_in[vt_i * P:(vt_i + 1) * P,
                                 vc * T:(vc + 1) * T],
                        in_=v_l)
            nc.gpsimd.collective_compute(
                "AllGather", mybir.AluOpType.bypass, replica_groups=groups,
                ins=[v_in.opt()], outs=[v_out.opt()])

            for ct in range(NHP):
                proj_qk(ct, qT[ct])

            # preload w_out during attention-adjacent window
            for it in range(NDT):
                nc.sync.dma_start(
                    out=wout_sb[it],
                    in_=wout_ext[it * P:(it + 1) * P, :].bitcast(F32R))

        # ---------------- Phase 3: attention ----------------
        n_batches = (NKT + EXP_BATCH - 1) // EXP_BATCH
        with tc.tile_pool(name="p3sb", bufs=4) as p3sb, \
             tc.tile_pool(name="p3pt", bufs=3) as p3pt, \
             tc.tile_pool(name="p3po", bufs=2, space="PSUM") as p3po, \
             tc.tile_pool(name="p3pd", bufs=2, space="PSUM") as p3pd:
            for hp in range(NHP):
                ps_o = [p3po.tile([65, T], F32, tag="po", name=f"po{hp}_{ab}") for ab in range(2)]
                for bi in range(n_batches):
                    kts = range(bi * EXP_BATCH,
                                min((bi + 1) * EXP_BATCH, NKT))
                    nb = len(kts)
                    pd = [p3pd.tile([P, EXP_BATCH, T], F32, tag="pd",
                                    name=f"pd{hp}_{bi}_{ab}")
                          for ab in range(2)]
                    vts = []
                    for i, kt in enumerate(kts):
                        c, w = kt // 4, kt % 4
                        k_slab = p3sb.tile([P, P], F32R, tag="ks")
                        nc.sync.dma_start(
                            out=k_slab,
                            in_=k_out[c * D + hp * P: c * D + (hp + 1) * P,
                                      w * P:(w + 1) * P].bitcast(F32R))
                        vt = p3sb.tile([P, 2, 65], F32R, tag="vt")
                        nc.sync.dma_start(
                            out=vt[:, :, 0:64],
                            in_=v_out[kt * P:(kt + 1) * P,
                                      hp * P:(hp + 1) * P]
                            .rearrange("p (h f) -> p h f", h=2).bitcast(F32R))
                        nc.vector.tensor_copy(out=vt[:, :, 64:65], in_=ones2f)
                        vts.append(vt)
                        for ab in range(2):
                            nc.tensor.matmul(
                                pd[ab][:, i, :],
                                k_slab[ab * 64:(ab + 1) * 64, :],
                                qT[hp][ab * 64:(ab + 1) * 64, :],
                                start=True, stop=True,
                                tile_position=(ab * 64, 0))
                    for ab in range(2):
                        pt = p3pt.tile([P, EXP_BATCH, T], F32R, tag="pt")
                        nc.scalar.activation(
                            out=pt[:, 0:nb, :], in_=pd[ab][:, 0:nb, :],
                            func=mybir.ActivationFunctionType.Exp,
                            scale=SCALE)
                        for i, kt in enumerate(kts):
                            nc.tensor.matmul(
                                ps_o[ab], vts[i][:, ab, :], pt[:, i, :],
                                start=(kt == 0), stop=(kt == NKT - 1))
                # normalize: out rows 0..63 scaled by 1/row64
                for ab in range(2):
                    h = 2 * hp + ab
                    recip_s = p3sb.tile([1, T], F32, tag="rc")
                    nc.vector.reciprocal(out=recip_s, in_=ps_o[ab][64:65, :])
                    nc.sync.dma_start(out=recip_d[h:h + 1, :], in_=recip_s)
                    recipB = p3sb.tile([64, T], F32, tag="rb")
                    rd = recip_d[h:h + 1, :]
                    nc.sync.dma_start(out=recipB, in_=bass.AP(
                        tensor=rd.tensor, offset=rd.offset,
                        ap=[[0, 64]] + rd.ap[1:]))
                    nc.vector.tensor_mul(
                        out=attnT[hp][ab * 64:(ab + 1) * 64, :],
                        in0=ps_o[ab][0:64, :], in1=recipB)

        # ---------------- Phase 4: output projection ----------------
        with tc.tile_pool(name="p4sb", bufs=3) as p4sb, \
             tc.tile_pool(name="p4ps", bufs=2, space="PSUM") as p4ps:
            for tt in range(NTT):
                for dc in range(2):
                    ps_y = p4ps.tile([P, T], F32, tag="py")
                    for it in range(NDT):
                        nc.tensor.matmul(
                            ps_y, attnT[it][:, tt * P:(tt + 1) * P],
                            wout_sb[it][:, dc * T:(dc + 1) * T],
                            start=(it == 0), stop=(it == NDT - 1))
                    y_s = p4sb.tile([P, T], F32, tag="y")
                    if apply_b_out:
                        nc.vector.tensor_add(
                            out=y_s, in0=ps_y,
                            in1=boutB[:, dc * T:(dc + 1) * T])
                    else:
                        nc.vector.tensor_copy(out=y_s, in_=ps_y)
                    nc.sync.dma_start(
                        out=out_ext[tt * P:(tt + 1) * P,
                                    dc * T:(dc + 1) * T],
                        in_=y_s)

    _split_multiwaits(nc)
    return nc


_CACHE = {}
LAST_RESULTS = None


def kernel(x, ln_gamma, ln_beta, w_qkv, w_out, b_out):
    global LAST_RESULTS
    _maybe_install_ntff_hook()

    x = np.ascontiguousarray(np.asarray(x, dtype=np.float32))
    ln_gamma = np.asarray(ln_gamma, dtype=np.float32).reshape(1, D)
    ln_beta = np.asarray(ln_beta, dtype=np.float32).reshape(1, D)
    w_qkv = np.ascontiguousarray(np.asarray(w_qkv, dtype=np.float32))
    w_out = np.ascontiguousarray(np.asarray(w_out, dtype=np.float32))
    b_out = np.asarray(b_out, dtype=np.float32).reshape(1, D)

    apply_ln_affine = not (np.all(ln_gamma == 1.0) and np.all(ln_beta == 0.0))
    apply_b_out = not np.all(b_out == 0.0)

    key = (apply_ln_affine, apply_b_out)
    if key not in _CACHE:
        _CACHE[key] = build(*key)
    nc = _CACHE[key]

    in_maps = []
    for c in range(8):
        b, t = c // 4, c % 4
        in_maps.append({
            "x": np.ascontiguousarray(x[b, t * T:(t + 1) * T, :]),
            "ln_gamma": ln_gamma,
            "ln_beta": ln_beta,
            "w_qkv": w_qkv,
            "w_out": w_out,
            "b_out": b_out,
        })

    trace = bool(os.environ.get("BASS_TRACE"))
    res = run_bass_kernel_spmd(nc, in_maps, core_ids=list(range(8)),
                               trace=trace)
    LAST_RESULTS = res

    y = np.empty((B, S, D), dtype=np.float32)
    for c in range(8):
        b, t = c // 4, c % 4
        y[b, t * T:(t + 1) * T, :] = res.results[c]["out"]
    return y


# revision 9
# speedup vs baseline: 1.2625x; 1.2625x over previous
"""Distributed Trainium2 attention-block kernel (8 NeuronCores).

Problem: y = LN(x) -> QKV -> 16-head attention (seq 2048, dh 64) -> out-proj.
x [2,2048,1024] f32.

Sharding: token-parallel. Core c handles batch c//4, token quarter c%4
(512 query tokens). Each core computes Q,K,V for its own 512 tokens
(all heads), AllGathers K^T and V within its 4-core batch group, then
runs attention for its 512 queries over the full 2048-token sequence
and the final projection. Output shards are disjoint -> no reduction.

All matmuls run in float32r (tf32-like: full bf16-rate on TensorE for
free-dim >= 256, ~1.5e-4 matmul rel err measured on HW). f32r tiles are
DMA'd straight from f32 DRAM via bitcast - the PE rounds on read, so no
cast passes are needed anywhere.

Attention per head: dots computed transposed (k on partitions, q free)
so softmax-exp'd probabilities feed PV directly as the moving operand;
PV's stationary is [V_tile | ones] (M=65) so the softmax denominator
accumulates in PSUM row 64 for free. exp (ScalarE) reads dots PSUM in
batches of 3 k-tiles to amortize ACTIVATE instruction overhead. No
max-subtraction: scaled dots are ~N(0,1) (LN'd x, w_qkv ~ N(0,1/d)),
max over all scores ~6 => exp <= ~500, safe in f32.
"""

import os
import numpy as np

import concourse.bass as bass
import concourse.tile as tile
from concourse import mybir
from concourse.bass_utils import run_bass_kernel_spmd
from concourse.masks import make_identity

F32 = mybir.dt.float32
F32R = mybir.dt.float32r

B, S, D = 2, 2048, 1024
H, DH = 16, 64
T = 512           # query tokens per core
P = 128
NKT = S // P      # 16 k-tiles
LN_EPS = 1e-5
SCALE = DH ** -0.5
EXP_BATCH = 3     # k-tiles per exp ACTIVATE call

_MAXW = 1


def _split_multiwaits(nc):
    """This container's walrus rejects >1 sync wait/update per instruction.
    Move extras onto adjacent same-engine NoOps."""
    import bass_rust

    for bb in nc.main_func.blocks:
        new_insts = []
        for inst in bb.instructions:
            si = inst.sync_info
            pre, post = [], []
            if si is not None:
                waits = list(si.on_wait or [])
                ups = list(si.on_update or [])
                if len(waits) > _MAXW or len(ups) > _MAXW:
                    for i in range(_MAXW, len(waits), _MAXW):
                        pre.append(bass_rust.InstNoOp(
                            name=f"I-{nc.next_id()}", engine=inst.engine,
                            ins=[], outs=[],
                            sync_info=mybir.SyncInfo(
                                on_wait=waits[i:i + _MAXW], on_update=[])))
                    for i in range(_MAXW, len(ups), _MAXW):
                        post.append(bass_rust.InstNoOp(
                            name=f"I-{nc.next_id()}", engine=inst.engine,
                            ins=[], outs=[],
                            sync_info=mybir.SyncInfo(
                                on_wait=[], on_update=ups[i:i + _MAXW])))
                    inst.sync_info = mybir.SyncInfo(
                        on_wait=waits[:_MAXW], on_update=ups[:_MAXW])
            new_insts.extend(pre)
            new_insts.append(inst)
            new_insts.extend(post)
        bb.instructions[:] = new_insts


def _maybe_install_ntff_hook():
    """Optional NTFF profiling support (BASS_TRACE=1); harmless if absent."""
    if not os.environ.get("BASS_TRACE"):
        return
    import sys
    import types
    if "antenv.axon_hooks" in sys.modules:
        return
    try:
        mod = types.ModuleType("antenv.axon_hooks")
        _h = [None]
        mod.set_axon_ntff_profile_hook = lambda h: _h.__setitem__(0, h)
        mod.get_axon_ntff_profile_hook = lambda: _h[0]
        import antenv
        from trn_agent_boot.trn_boot import _ntff_profile_via_ctypes
        hook = _ntff_profile_via_ctypes('/opt/axon/libaxon_pjrt.so')
        sys.modules["antenv.axon_hooks"] = mod
        antenv.axon_hooks = mod
        mod.set_axon_ntff_profile_hook(hook)
    except Exception:
        pass


def build(apply_ln_affine, apply_b_out):
    nc = bass.Bass()

    x_ext = nc.declare_dram_parameter("x", [T, D], F32, isOutput=False)
    gamma_ext = nc.declare_dram_parameter("ln_gamma", [1, D], F32, isOutput=False)
    beta_ext = nc.declare_dram_parameter("ln_beta", [1, D], F32, isOutput=False)
    wqkv_ext = nc.declare_dram_parameter("w_qkv", [D, 3 * D], F32, isOutput=False)
    wout_ext = nc.declare_dram_parameter("w_out", [D, D], F32, isOutput=False)
    bout_ext = nc.declare_dram_parameter("b_out", [1, D], F32, isOutput=False)
    out_ext = nc.declare_dram_parameter("out", [T, D], F32, isOutput=True)

    groups = [[0, 1, 2, 3], [4, 5, 6, 7]]
    NDT = D // P   # 8 contraction tiles over model dim
    NTT = T // P   # 4 token tiles per core
    NHP = H // 2   # 8 head pairs

    from contextlib import ExitStack
    with tile.TileContext(nc) as tc, ExitStack() as stack:
        consts = stack.enter_context(tc.tile_pool(name="consts", bufs=1))
        sb_main = stack.enter_context(tc.tile_pool(name="sb_main", bufs=1))
        dram = stack.enter_context(tc.tile_pool(name="dram", bufs=1, space="DRAM"))

        ident = consts.tile([P, P], F32)
        make_identity(nc, ident)
        eps_t = consts.tile([P, 1], F32)
        nc.vector.memset(eps_t, LN_EPS)
        ones8 = consts.tile([P, 8], F32)
        nc.vector.memset(ones8, 1.0)

        if apply_ln_affine:
            gammaB = consts.tile([P, D], F32)
            betaB = consts.tile([P, D], F32)
            nc.sync.dma_start(out=gammaB, in_=bass.AP(
                tensor=gamma_ext.tensor, offset=gamma_ext.offset,
                ap=[[0, P]] + gamma_ext.ap[1:]))
            nc.sync.dma_start(out=betaB, in_=bass.AP(
                tensor=beta_ext.tensor, offset=beta_ext.offset,
                ap=[[0, P]] + beta_ext.ap[1:]))
        if apply_b_out:
            boutB = consts.tile([P, D], F32)
            nc.sync.dma_start(out=boutB, in_=bass.AP(
                tensor=bout_ext.tensor, offset=bout_ext.offset,
                ap=[[0, P]] + bout_ext.ap[1:]))

        # persistent activations
        xnT = [sb_main.tile([P, T], F32R, tag=f"xnT{i}", name=f"xnT{i}") for i in range(NDT)]
        qT = [sb_main.tile([P, T], F32R, tag=f"qT{i}", name=f"qT{i}") for i in range(NHP)]
        attnT = [sb_main.tile([P, T], F32R, tag=f"attnT{i}", name=f"attnT{i}") for i in range(NHP)]
        wout_sb = [sb_main.tile([P, D], F32R, tag=f"wout{i}", name=f"wout{i}") for i in range(NDT)]

        # AG buffers (internal DRAM), split in two (hp 0-3 / hp 4-7) so each
        # collective stays under the ~1MB mesh-algorithm crossover and
        # overlaps with projection / attention of the other half.
        # v buffers are augmented: per head, 64 value columns + 1 ones
        # column (so PV's stationary [V|1] reads are contiguous post-AG).
        VA = 2 * 65  # 130 cols per head-pair in augmented v
        k_in2 = [dram.tile([T, T], F32, name=f"k_in{g}") for g in range(2)]
        k_out2 = [dram.tile([4 * T, T], F32, name=f"k_out{g}") for g in range(2)]
        v_in2 = [dram.tile([T, 4 * VA], F32, name=f"v_in{g}") for g in range(2)]
        v_out2 = [dram.tile([S, 4 * VA], F32, name=f"v_out{g}") for g in range(2)]
        recip_d = dram.tile([H, T], F32)

        # ---------------- Phase 1: LayerNorm + transpose ----------------
        with tc.tile_pool(name="p1sb", bufs=3) as p1sb, \
             tc.tile_pool(name="p1ps", bufs=4, space="PSUM") as p1ps:
            for tt in range(NTT):
                x_t = p1sb.tile([P, D], F32, tag="x")
                nc.sync.dma_start(out=x_t, in_=x_ext[tt * P:(tt + 1) * P, :])
                stats = p1sb.tile([P, 2, nc.vector.BN_STATS_DIM], F32, tag="st")
                for sg in range(2):
                    nc.vector.bn_stats(out=stats[:, sg, :],
                                       in_=x_t[:, sg * 512:(sg + 1) * 512])
                mv = p1sb.tile([P, nc.vector.BN_AGGR_DIM], F32, tag="mv")
                nc.vector.bn_aggr(out=mv, in_=stats)
                rstd = p1sb.tile([P, 1], F32, tag="rstd")
                nc.scalar.activation(out=rstd, in_=mv[:, 1:2],
                                     func=mybir.ActivationFunctionType.Sqrt,
                                     bias=eps_t, scale=1.0)
                nc.vector.reciprocal(out=rstd, in_=rstd)
                xn_t = p1sb.tile([P, D], F32, tag="xn")
                nc.vector.tensor_scalar(
                    out=xn_t, in0=x_t, scalar1=mv[:, 0:1], scalar2=rstd,
                    op0=mybir.AluOpType.subtract, op1=mybir.AluOpType.mult)
                if apply_ln_affine:
                    nc.vector.tensor_mul(out=xn_t, in0=xn_t, in1=gammaB)
                    nc.vector.tensor_add(out=xn_t, in0=xn_t, in1=betaB)
                for dt in range(NDT):
                    ps_tr = p1ps.tile([P, P], F32, tag="tr")
                    nc.tensor.transpose(ps_tr, xn_t[:, dt * P:(dt + 1) * P], ident)
                    nc.vector.tensor_copy(out=xnT[dt][:, tt * P:(tt + 1) * P],
                                          in_=ps_tr)

        # ---------------- Phase 2: QKV projection + AllGathers ----------------
        # w_qkv loaded as 8 contiguous row-slabs [128, 3072] (128 descriptors
        # each instead of per-element-column striding).
        with tc.tile_pool(name="p2w", bufs=1) as p2w, \
             tc.tile_pool(name="p2sb", bufs=4) as p2sb, \
             tc.tile_pool(name="p2ps", bufs=4, space="PSUM") as p2ps:
            wslab = []
            for dt in range(NDT):
                w_s = p2w.tile([P, 3 * D], F32R, tag=f"ws{dt}", name=f"ws{dt}")
                nc.sync.dma_start(
                    out=w_s,
                    in_=wqkv_ext[dt * P:(dt + 1) * P, :].bitcast(F32R))
                wslab.append(w_s)

            def proj_colT(fbase, dst):
                # dst[f, tok] = sum_d w[d, fbase+f] * xnT[d, tok], f in 0..127
                ps = p2ps.tile([P, T], F32, tag="pqk")
                for dt in range(NDT):
                    nc.tensor.matmul(ps, wslab[dt][:, fbase:fbase + P],
                                     xnT[dt],
                                     start=(dt == 0), stop=(dt == NDT - 1))
                nc.vector.tensor_copy(out=dst, in_=ps)

            def proj_k_group(g):
                for i in range(4):
                    ct = 4 * g + i
                    kt_l = p2sb.tile([P, T], F32R, tag="ktl")
                    proj_colT(D + ct * P, kt_l)
                    nc.sync.dma_start(
                        out=k_in2[g][i * P:(i + 1) * P, :],
                        in_=kt_l.bitcast(F32))
                nc.gpsimd.collective_compute(
                    "AllGather", mybir.AluOpType.bypass,
                    replica_groups=groups,
                    ins=[k_in2[g].opt()], outs=[k_out2[g].opt()])

            def proj_v_group(g):
                # v natural [tok, feat] for head-pairs 4g..4g+3, with a ones
                # column interleaved after each head's 64 value columns.
                for vt_i in range(NTT):
                    ps = p2ps.tile([P, T], F32, tag="pv")
                    for dt in range(NDT):
                        nc.tensor.matmul(
                            ps, xnT[dt][:, vt_i * P:(vt_i + 1) * P],
                            wslab[dt][:, 2 * D + g * T: 2 * D + (g + 1) * T],
                            start=(dt == 0), stop=(dt == NDT - 1))
                    v_l = p2sb.tile([P, 8, 65], F32, tag="vl")
                    nc.vector.tensor_copy(
                        out=v_l[:, :, 0:64],
                        in_=ps.rearrange("p (h f) -> p h f", h=8))
                    nc.vector.tensor_copy(
                        out=v_l[:, :, 64:65],
                        in_=ones8.rearrange("p (h o) -> p h o", h=8))
                    nc.sync.dma_start(
                        out=v_in2[g][vt_i * P:(vt_i + 1) * P, :],
                        in_=v_l.rearrange("p h f -> p (h f)"))
                nc.gpsimd.collective_compute(
                    "AllGather", mybir.AluOpType.bypass,
                    replica_groups=groups,
                    ins=[v_in2[g].opt()], outs=[v_out2[g].opt()])

            proj_k_group(0)
            proj_v_group(0)
            proj_k_group(1)
            proj_v_group(1)
            for ct in range(NHP):
                proj_colT(ct * P, qT[ct])

            # preload w_out during attention-adjacent window
            for it in range(NDT):
                nc.sync.dma_start(
                    out=wout_sb[it],
                    in_=wout_ext[it * P:(it + 1) * P, :].bitcast(F32R))

        # ---------------- Phase 3: attention ----------------
        n_batches = (NKT + EXP_BATCH - 1) // EXP_BATCH
        with tc.tile_pool(name="p3sb", bufs=3) as p3sb, \
             tc.tile_pool(name="p3o", bufs=4) as p3o, \
             tc.tile_pool(name="p3pt", bufs=3) as p3pt, \
             tc.tile_pool(name="p3po", bufs=2, space="PSUM") as p3po, \
             tc.tile_pool(name="p3pd", bufs=2, space="PSUM") as p3pd:
            for hp in range(NHP):
                g, hq = hp // 4, hp % 4
                ps_o = [p3po.tile([65, T], F32, tag="po", name=f"po{hp}_{ab}")
                        for ab in range(2)]
                # k slabs: [128, 512] contiguous rows, one per token-chunk
                kslabs = []
                for c in range(4):
                    ks = p3sb.tile([P, T], F32R, tag="ks", name=f"ks{hp}_{c}")
                    nc.sync.dma_start(
                        out=ks,
                        in_=k_out2[g][c * T + hq * P: c * T + (hq + 1) * P, :]
                        .bitcast(F32R))
                    kslabs.append(ks)
                for bi in range(n_batches):
                    kts = range(bi * EXP_BATCH,
                                min((bi + 1) * EXP_BATCH, NKT))
                    nb = len(kts)
                    pd = [p3pd.tile([P, EXP_BATCH, T], F32, tag="pd",
                                    name=f"pd{hp}_{bi}_{ab}")
                          for ab in range(2)]
                    vts = []
                    for i, kt in enumerate(kts):
                        c, w = kt // 4, kt % 4
                        vt = p3sb.tile([P, 2, 65], F32R, tag="vt",
                                       name=f"vt{hp}_{kt}")
                        nc.sync.dma_start(
                            out=vt,
                            in_=v_out2[g][kt * P:(kt + 1) * P,
                                          hq * VA:(hq + 1) * VA]
                            .rearrange("p (h f) -> p h f", h=2).bitcast(F32R))
                        vts.append(vt)
                        for ab in range(2):
                            nc.tensor.matmul(
                                pd[ab][:, i, :],
                                kslabs[c][ab * 64:(ab + 1) * 64,
                                          w * P:(w + 1) * P],
                                qT[hp][ab * 64:(ab + 1) * 64, :],
                                start=True, stop=True,
                                tile_position=(ab * 64, 0))
                    for ab in range(2):
                        pt = p3pt.tile([P, EXP_BATCH, T], F32R, tag="pt")
                        nc.scalar.activation(
                            out=pt[:, 0:nb, :], in_=pd[ab][:, 0:nb, :],
                            func=mybir.ActivationFunctionType.Exp,
                            scale=SCALE)
                        for i, kt in enumerate(kts):
                            nc.tensor.matmul(
                                ps_o[ab], vts[i][:, ab, :], pt[:, i, :],
                                start=(kt == 0), stop=(kt == NKT - 1))
                # drain PSUM accumulators to SBUF immediately (frees po for
                # the next head pair); normalization happens off-path below
                for ab in range(2):
                    h = 2 * hp + ab
                    o_raw = p3o.tile([65, T], F32, tag="oraw",
                                     name=f"oraw{h}")
                    nc.vector.tensor_copy(out=o_raw, in_=ps_o[ab])
                    nc.sync.dma_start(out=recip_d[h:h + 1, :],
                                      in_=o_raw[64:65, :])
                    sumB = p3sb.tile([64, T], F32, tag="rb")
                    rd = recip_d[h:h + 1, :]
                    nc.sync.dma_start(out=sumB, in_=bass.AP(
                        tensor=rd.tensor, offset=rd.offset,
                        ap=[[0, 64]] + rd.ap[1:]))
                    nc.vector.reciprocal(out=sumB, in_=sumB)
                    nc.vector.tensor_mul(
                        out=attnT[hp][ab * 64:(ab + 1) * 64, :],
                        in0=o_raw[0:64, :], in1=sumB)

        # ---------------- Phase 4: output projection ----------------
        with tc.tile_pool(name="p4sb", bufs=3) as p4sb, \
             tc.tile_pool(name="p4ps", bufs=2, space="PSUM") as p4ps:
            for tt in range(NTT):
                for dc in range(2):
                    ps_y = p4ps.tile([P, T], F32, tag="py")
                    for it in range(NDT):
                        nc.tensor.matmul(
                            ps_y, attnT[it][:, tt * P:(tt + 1) * P],
                            wout_sb[it][:, dc * T:(dc + 1) * T],
                            start=(it == 0), stop=(it == NDT - 1))
                    y_s = p4sb.tile([P, T], F32, tag="y")
                    if apply_b_out:
                        nc.vector.tensor_add(
                            out=y_s, in0=ps_y,
                            in1=boutB[:, dc * T:(dc + 1) * T])
                    else:
                        nc.vector.tensor_copy(out=y_s, in_=ps_y)
                    nc.sync.dma_start(
                        out=out_ext[tt * P:(tt + 1) * P,
                                    dc * T:(dc + 1) * T],
                        in_=y_s)

    _split_multiwaits(nc)
    return nc


_CACHE = {}
LAST_RESULTS = None


def kernel(x, ln_gamma, ln_beta, w_qkv, w_out, b_out):
    global LAST_RESULTS
    _maybe_install_ntff_hook()

    x = np.ascontiguousarray(np.asarray(x, dtype=np.float32))
    ln_gamma = np.asarray(ln_gamma, dtype=np.float32).reshape(1, D)
    ln_beta = np.asarray(ln_beta, dtype=np.float32).reshape(1, D)
    w_qkv = np.ascontiguousarray(np.asarray(w_qkv, dtype=np.float32))
    w_out = np.ascontiguousarray(np.asarray(w_out, dtype=np.float32))
    b_out = np.asarray(b_out, dtype=np.float32).reshape(1, D)

    apply_ln_affine = not (np.all(ln_gamma == 1.0) and np.all(ln_beta == 0.0))
    apply_b_out = not np.all(b_out == 0.0)

    key = (apply_ln_affine, apply_b_out)
    if key not in _CACHE:
        _CACHE[key] = build(*key)
    nc = _CACHE[key]

    in_maps = []
    for c in range(8):
        b, t = c // 4, c % 4
        in_maps.append({
            "x": np.ascontiguousarray(x[b, t * T:(t + 1) * T, :]),
            "ln_gamma": ln_gamma,
            "ln_beta": ln_beta,
            "w_qkv": w_qkv,
            "w_out": w_out,
            "b_out": b_out,
        })

    trace = bool(os.environ.get("BASS_TRACE"))
    res = run_bass_kernel_spmd(nc, in_maps, core_ids=list(range(8)),
                               trace=trace)
    LAST_RESULTS = res

    y = np.empty((B, S, D), dtype=np.float32)
    for c in range(8):
        b, t = c // 4, c % 4
        y[b, t * T:(t + 1) * T, :] = res.results[c]["out"]
    return y


# revision 11
# speedup vs baseline: 1.7096x; 1.3541x over previous
"""Distributed Trainium2 attention-block kernel (8 NeuronCores).

Problem: y = LN(x) -> QKV -> 16-head attention (seq 2048, dh 64) -> out-proj.
x [2,2048,1024] f32.

Sharding: token-parallel. Core c handles batch c//4, token quarter c%4
(512 query tokens). Each core computes Q,K,V for its own 512 tokens
(all heads), AllGathers K^T and V within its 4-core batch group, then
runs attention for its 512 queries over the full 2048-token sequence
and the final projection. Output shards are disjoint -> no reduction.

All matmuls run in float32r (tf32-like: full bf16-rate on TensorE for
free-dim >= 256, ~1.5e-4 matmul rel err measured on HW). f32r tiles are
DMA'd straight from f32 DRAM via bitcast - the PE rounds on read, so no
cast passes are needed anywhere.

Attention per head: dots computed transposed (k on partitions, q free)
so softmax-exp'd probabilities feed PV directly as the moving operand;
PV's stationary is [V_tile | ones] (M=65) so the softmax denominator
accumulates in PSUM row 64 for free. exp (ScalarE) reads dots PSUM in
batches of 3 k-tiles to amortize ACTIVATE instruction overhead. No
max-subtraction: scaled dots are ~N(0,1) (LN'd x, w_qkv ~ N(0,1/d)),
max over all scores ~6 => exp <= ~500, safe in f32.
"""

import os
import numpy as np

import concourse.bass as bass
import concourse.tile as tile
from concourse import mybir
from concourse.bass_utils import run_bass_kernel_spmd
from concourse.masks import make_identity

F32 = mybir.dt.float32
F32R = mybir.dt.float32r
BF16 = mybir.dt.bfloat16

B, S, D = 2, 2048, 1024
H, DH = 16, 64
T = 512           # query tokens per core
P = 128
NKT = S // P      # 16 k-tiles
LN_EPS = 1e-5
SCALE = DH ** -0.5
EXP_BATCH = 3     # k-tiles per exp ACTIVATE call

_MAXW = 1


def _split_multiwaits(nc):
    """This container's walrus rejects >1 sync wait/update per instruction.
    Move extras onto adjacent same-engine NoOps."""
    import bass_rust

    for bb in nc.main_func.blocks:
        new_insts = []
        for inst in bb.instructions:
            si = inst.sync_info
            pre, post = [], []
            if si is not None:
                waits = list(si.on_wait or [])
                ups = list(si.on_update or [])
                if len(waits) > _MAXW or len(ups) > _MAXW:
                    for i in range(_MAXW, len(waits), _MAXW):
                        pre.append(bass_rust.InstNoOp(
                            name=f"I-{nc.next_id()}", engine=inst.engine,
                            ins=[], outs=[],
                            sync_info=mybir.SyncInfo(
                                on_wait=waits[i:i + _MAXW], on_update=[])))
                    for i in range(_MAXW, len(ups), _MAXW):
                        post.append(bass_rust.InstNoOp(
                            name=f"I-{nc.next_id()}", engine=inst.engine,
                            ins=[], outs=[],
                            sync_info=mybir.SyncInfo(
                                on_wait=[], on_update=ups[i:i + _MAXW])))
                    inst.sync_info = mybir.SyncInfo(
                        on_wait=waits[:_MAXW], on_update=ups[:_MAXW])
            new_insts.extend(pre)
            new_insts.append(inst)
            new_insts.extend(post)
        bb.instructions[:] = new_insts


def _maybe_install_ntff_hook():
    """Optional NTFF profiling support (BASS_TRACE=1); harmless if absent."""
    if not os.environ.get("BASS_TRACE"):
        return
    import sys
    import types
    if "antenv.axon_hooks" in sys.modules:
        return
    try:
        mod = types.ModuleType("antenv.axon_hooks")
        _h = [None]
        mod.set_axon_ntff_profile_hook = lambda h: _h.__setitem__(0, h)
        mod.get_axon_ntff_profile_hook = lambda: _h[0]
        import antenv
        from trn_agent_boot.trn_boot import _ntff_profile_via_ctypes
        hook = _ntff_profile_via_ctypes('/opt/axon/libaxon_pjrt.so')
        sys.modules["antenv.axon_hooks"] = mod
        antenv.axon_hooks = mod
        mod.set_axon_ntff_profile_hook(hook)
    except Exception:
        pass


def build(apply_ln_affine, apply_b_out):
    nc = bass.Bass()

    x_ext = nc.declare_dram_parameter("x", [T, D], F32, isOutput=False)
    gamma_ext = nc.declare_dram_parameter("ln_gamma", [1, D], F32, isOutput=False)
    beta_ext = nc.declare_dram_parameter("ln_beta", [1, D], F32, isOutput=False)
    wqkv_ext = nc.declare_dram_parameter("w_qkv", [D, 3 * D], F32, isOutput=False)
    wout_ext = nc.declare_dram_parameter("w_out", [D, D], F32, isOutput=False)
    bout_ext = nc.declare_dram_parameter("b_out", [1, D], F32, isOutput=False)
    out_ext = nc.declare_dram_parameter("out", [T, D], F32, isOutput=True)

    groups = [[0, 1, 2, 3], [4, 5, 6, 7]]
    NDT = D // P   # 8 contraction tiles over model dim
    NTT = T // P   # 4 token tiles per core
    NHP = H // 2   # 8 head pairs

    from contextlib import ExitStack
    with tile.TileContext(nc) as tc, ExitStack() as stack:
        consts = stack.enter_context(tc.tile_pool(name="consts", bufs=1))
        sb_main = stack.enter_context(tc.tile_pool(name="sb_main", bufs=1))
        dram = stack.enter_context(tc.tile_pool(name="dram", bufs=1, space="DRAM"))

        ident = consts.tile([P, P], F32)
        make_identity(nc, ident)
        eps_t = consts.tile([P, 1], F32)
        nc.vector.memset(eps_t, LN_EPS)
        ones8 = consts.tile([P, 8], F32)
        nc.vector.memset(ones8, 1.0)

        if apply_ln_affine:
            gammaB = consts.tile([P, D], F32)
            betaB = consts.tile([P, D], F32)
            nc.sync.dma_start(out=gammaB, in_=bass.AP(
                tensor=gamma_ext.tensor, offset=gamma_ext.offset,
                ap=[[0, P]] + gamma_ext.ap[1:]))
            nc.sync.dma_start(out=betaB, in_=bass.AP(
                tensor=beta_ext.tensor, offset=beta_ext.offset,
                ap=[[0, P]] + beta_ext.ap[1:]))
        if apply_b_out:
            boutB = consts.tile([P, D], F32)
            nc.sync.dma_start(out=boutB, in_=bass.AP(
                tensor=bout_ext.tensor, offset=bout_ext.offset,
                ap=[[0, P]] + bout_ext.ap[1:]))

        # persistent activations
        xnT = [sb_main.tile([P, T], F32R, tag=f"xnT{i}", name=f"xnT{i}") for i in range(NDT)]
        qT = [sb_main.tile([P, T], BF16, tag=f"qT{i}", name=f"qT{i}") for i in range(NHP)]
        attnT = [sb_main.tile([P, T], F32R, tag=f"attnT{i}", name=f"attnT{i}") for i in range(NHP)]
        wout_sb = [sb_main.tile([P, D], F32R, tag=f"wout{i}", name=f"wout{i}") for i in range(NDT)]

        # AG buffers (internal DRAM), split in two (hp 0-3 / hp 4-7) so each
        # collective stays under the ~1MB mesh-algorithm crossover and
        # overlaps with projection / attention of the other half.
        # v buffers are augmented: per head, 64 value columns + 1 ones
        # column (so PV's stationary [V|1] reads are contiguous post-AG).
        VA = 2 * 65  # 130 cols per head-pair in augmented v
        k_in2 = [dram.tile([T, T], BF16, name=f"k_in{g}") for g in range(2)]
        k_out2 = [dram.tile([4 * T, T], BF16, name=f"k_out{g}") for g in range(2)]
        v_in2 = [dram.tile([T, 4 * VA], BF16, name=f"v_in{g}") for g in range(2)]
        v_out2 = [dram.tile([S, 4 * VA], BF16, name=f"v_out{g}") for g in range(2)]
        recip_d = dram.tile([H, T], F32)

        # ---------------- Phase 1: LayerNorm + transpose ----------------
        with tc.tile_pool(name="p1sb", bufs=3) as p1sb, \
             tc.tile_pool(name="p1ps", bufs=4, space="PSUM") as p1ps:
            for tt in range(NTT):
                x_t = p1sb.tile([P, D], F32, tag="x")
                nc.sync.dma_start(out=x_t, in_=x_ext[tt * P:(tt + 1) * P, :])
                stats = p1sb.tile([P, 2, nc.vector.BN_STATS_DIM], F32, tag="st")
                for sg in range(2):
                    nc.vector.bn_stats(out=stats[:, sg, :],
                                       in_=x_t[:, sg * 512:(sg + 1) * 512])
                mv = p1sb.tile([P, nc.vector.BN_AGGR_DIM], F32, tag="mv")
                nc.vector.bn_aggr(out=mv, in_=stats)
                rstd = p1sb.tile([P, 1], F32, tag="rstd")
                nc.scalar.activation(out=rstd, in_=mv[:, 1:2],
                                     func=mybir.ActivationFunctionType.Sqrt,
                                     bias=eps_t, scale=1.0)
                nc.vector.reciprocal(out=rstd, in_=rstd)
                xn_t = p1sb.tile([P, D], F32, tag="xn")
                nc.vector.tensor_scalar(
                    out=xn_t, in0=x_t, scalar1=mv[:, 0:1], scalar2=rstd,
                    op0=mybir.AluOpType.subtract, op1=mybir.AluOpType.mult)
                if apply_ln_affine:
                    nc.vector.tensor_mul(out=xn_t, in0=xn_t, in1=gammaB)
                    nc.vector.tensor_add(out=xn_t, in0=xn_t, in1=betaB)
                for dt in range(NDT):
                    ps_tr = p1ps.tile([P, P], F32, tag="tr")
                    nc.tensor.transpose(ps_tr, xn_t[:, dt * P:(dt + 1) * P], ident)
                    nc.vector.tensor_copy(out=xnT[dt][:, tt * P:(tt + 1) * P],
                                          in_=ps_tr)

        # ---------------- Phase 2: QKV projection + AllGathers ----------------
        # w_qkv loaded as 8 contiguous row-slabs [128, 3072] (128 descriptors
        # each instead of per-element-column striding).
        with tc.tile_pool(name="p2w", bufs=1) as p2w, \
             tc.tile_pool(name="p2sb", bufs=4) as p2sb, \
             tc.tile_pool(name="p2ps", bufs=4, space="PSUM") as p2ps:
            wslab = []
            for dt in range(NDT):
                w_s = p2w.tile([P, 3 * D], F32R, tag=f"ws{dt}", name=f"ws{dt}")
                nc.sync.dma_start(
                    out=w_s,
                    in_=wqkv_ext[dt * P:(dt + 1) * P, :].bitcast(F32R))
                wslab.append(w_s)

            def proj_colT(fbase, dst):
                # dst[f, tok] = sum_d w[d, fbase+f] * xnT[d, tok], f in 0..127
                ps = p2ps.tile([P, T], F32, tag="pqk")
                for dt in range(NDT):
                    nc.tensor.matmul(ps, wslab[dt][:, fbase:fbase + P],
                                     xnT[dt],
                                     start=(dt == 0), stop=(dt == NDT - 1))
                nc.vector.tensor_copy(out=dst, in_=ps)

            def proj_k_group(g):
                for i in range(4):
                    ct = 4 * g + i
                    kt_l = p2sb.tile([P, T], BF16, tag="ktl")
                    proj_colT(D + ct * P, kt_l)
                    nc.sync.dma_start(
                        out=k_in2[g][i * P:(i + 1) * P, :],
                        in_=kt_l)
                nc.gpsimd.collective_compute(
                    "AllGather", mybir.AluOpType.bypass,
                    replica_groups=groups,
                    ins=[k_in2[g].opt()], outs=[k_out2[g].opt()])

            def proj_v_group(g):
                # v natural [tok, feat] for head-pairs 4g..4g+3, with a ones
                # column interleaved after each head's 64 value columns.
                for vt_i in range(NTT):
                    ps = p2ps.tile([P, T], F32, tag="pv")
                    for dt in range(NDT):
                        nc.tensor.matmul(
                            ps, xnT[dt][:, vt_i * P:(vt_i + 1) * P],
                            wslab[dt][:, 2 * D + g * T: 2 * D + (g + 1) * T],
                            start=(dt == 0), stop=(dt == NDT - 1))
                    v_l = p2sb.tile([P, 8, 65], BF16, tag="vl")
                    nc.vector.tensor_copy(
                        out=v_l[:, :, 0:64],
                        in_=ps.rearrange("p (h f) -> p h f", h=8))
                    nc.vector.tensor_copy(
                        out=v_l[:, :, 64:65],
                        in_=ones8.rearrange("p (h o) -> p h o", h=8))
                    nc.sync.dma_start(
                        out=v_in2[g][vt_i * P:(vt_i + 1) * P, :],
                        in_=v_l.rearrange("p h f -> p (h f)"))
                nc.gpsimd.collective_compute(
                    "AllGather", mybir.AluOpType.bypass,
                    replica_groups=groups,
                    ins=[v_in2[g].opt()], outs=[v_out2[g].opt()])

            proj_k_group(0)
            proj_v_group(0)
            proj_k_group(1)
            proj_v_group(1)
            for ct in range(NHP):
                proj_colT(ct * P, qT[ct])

            # preload w_out during attention-adjacent window
            for it in range(NDT):
                nc.sync.dma_start(
                    out=wout_sb[it],
                    in_=wout_ext[it * P:(it + 1) * P, :].bitcast(F32R))

        # ---------------- Phase 3: attention ----------------
        n_batches = (NKT + EXP_BATCH - 1) // EXP_BATCH
        with tc.tile_pool(name="p3sb", bufs=3) as p3sb, \
             tc.tile_pool(name="p3o", bufs=4) as p3o, \
             tc.tile_pool(name="p3pt", bufs=3) as p3pt, \
             tc.tile_pool(name="p3po", bufs=2, space="PSUM") as p3po, \
             tc.tile_pool(name="p3pd", bufs=2, space="PSUM") as p3pd:
            for hp in range(NHP):
                g, hq = hp // 4, hp % 4
                ps_o = [p3po.tile([65, T], F32, tag="po", name=f"po{hp}_{ab}")
                        for ab in range(2)]
                # k slabs: [128, 512] contiguous rows, one per token-chunk
                kslabs = []
                for c in range(4):
                    ks = p3sb.tile([P, T], BF16, tag="ks", name=f"ks{hp}_{c}",
                                   bufs=8)
                    nc.sync.dma_start(
                        out=ks,
                        in_=k_out2[g][c * T + hq * P: c * T + (hq + 1) * P, :])
                    kslabs.append(ks)
                for bi in range(n_batches):
                    kts = range(bi * EXP_BATCH,
                                min((bi + 1) * EXP_BATCH, NKT))
                    nb = len(kts)
                    pd = [p3pd.tile([P, EXP_BATCH, T], F32, tag="pd",
                                    name=f"pd{hp}_{bi}_{ab}")
                          for ab in range(2)]
                    vts = []
                    for i, kt in enumerate(kts):
                        c, w = kt // 4, kt % 4
                        vt = p3sb.tile([P, 2, 65], BF16, tag="vt",
                                       name=f"vt{hp}_{kt}", bufs=6)
                        nc.sync.dma_start(
                            out=vt,
                            in_=v_out2[g][kt * P:(kt + 1) * P,
                                          hq * VA:(hq + 1) * VA]
                            .rearrange("p (h f) -> p h f", h=2))
                        vts.append(vt)
                        for ab in range(2):
                            nc.tensor.matmul(
                                pd[ab][:, i, :],
                                kslabs[c][ab * 64:(ab + 1) * 64,
                                          w * P:(w + 1) * P],
                                qT[hp][ab * 64:(ab + 1) * 64, :],
                                start=True, stop=True,
                                tile_position=(ab * 64, 0))
                    for ab in range(2):
                        pt = p3pt.tile([P, EXP_BATCH, T], BF16, tag="pt", bufs=4)
                        nc.scalar.activation(
                            out=pt[:, 0:nb, :], in_=pd[ab][:, 0:nb, :],
                            func=mybir.ActivationFunctionType.Exp,
                            scale=SCALE)
                        for i, kt in enumerate(kts):
                            nc.tensor.matmul(
                                ps_o[ab], vts[i][:, ab, :], pt[:, i, :],
                                start=(kt == 0), stop=(kt == NKT - 1))
                # drain PSUM accumulators to SBUF immediately (frees po for
                # the next head pair); normalization happens off-path below
                for ab in range(2):
                    h = 2 * hp + ab
                    o_raw = p3o.tile([65, T], F32, tag="oraw",
                                     name=f"oraw{h}")
                    nc.vector.tensor_copy(out=o_raw, in_=ps_o[ab])
                    nc.sync.dma_start(out=recip_d[h:h + 1, :],
                                      in_=o_raw[64:65, :])
                    sumB = p3sb.tile([64, T], F32, tag="rb")
                    rd = recip_d[h:h + 1, :]
                    nc.sync.dma_start(out=sumB, in_=bass.AP(
                        tensor=rd.tensor, offset=rd.offset,
                        ap=[[0, 64]] + rd.ap[1:]))
                    nc.vector.reciprocal(out=sumB, in_=sumB)
                    nc.vector.tensor_mul(
                        out=attnT[hp][ab * 64:(ab + 1) * 64, :],
                        in0=o_raw[0:64, :], in1=sumB)

        # ---------------- Phase 4: output projection ----------------
        with tc.tile_pool(name="p4sb", bufs=3) as p4sb, \
             tc.tile_pool(name="p4ps", bufs=2, space="PSUM") as p4ps:
            for tt in range(NTT):
                for dc in range(2):
                    ps_y = p4ps.tile([P, T], F32, tag="py")
                    for it in range(NDT):
                        nc.tensor.matmul(
                            ps_y, attnT[it][:, tt * P:(tt + 1) * P],
                            wout_sb[it][:, dc * T:(dc + 1) * T],
                            start=(it == 0), stop=(it == NDT - 1))
                    y_s = p4sb.tile([P, T], F32, tag="y")
                    if apply_b_out:
                        nc.vector.tensor_add(
                            out=y_s, in0=ps_y,
                            in1=boutB[:, dc * T:(dc + 1) * T])
                    else:
                        nc.vector.tensor_copy(out=y_s, in_=ps_y)
                    nc.sync.dma_start(
                        out=out_ext[tt * P:(tt + 1) * P,
                                    dc * T:(dc + 1) * T],
                        in_=y_s)

    _split_multiwaits(nc)
    return nc


_CACHE = {}
LAST_RESULTS = None


def kernel(x, ln_gamma, ln_beta, w_qkv, w_out, b_out):
    global LAST_RESULTS
    _maybe_install_ntff_hook()

    x = np.ascontiguousarray(np.asarray(x, dtype=np.float32))
    ln_gamma = np.asarray(ln_gamma, dtype=np.float32).reshape(1, D)
    ln_beta = np.asarray(ln_beta, dtype=np.float32).reshape(1, D)
    w_qkv = np.ascontiguousarray(np.asarray(w_qkv, dtype=np.float32))
    w_out = np.ascontiguousarray(np.asarray(w_out, dtype=np.float32))
    b_out = np.asarray(b_out, dtype=np.float32).reshape(1, D)

    apply_ln_affine = not (np.all(ln_gamma == 1.0) and np.all(ln_beta == 0.0))
    apply_b_out = not np.all(b_out == 0.0)

    key = (apply_ln_affine, apply_b_out)
    if key not in _CACHE:
        _CACHE[key] = build(*key)
    nc = _CACHE[key]

    in_maps = []
    for c in range(8):
        b, t = c // 4, c % 4
        in_maps.append({
            "x": np.ascontiguousarray(x[b, t * T:(t + 1) * T, :]),
            "ln_gamma": ln_gamma,
            "ln_beta": ln_beta,
            "w_qkv": w_qkv,
            "w_out": w_out,
            "b_out": b_out,
        })

    trace = bool(os.environ.get("BASS_TRACE"))
    res = run_bass_kernel_spmd(nc, in_maps, core_ids=list(range(8)),
                               trace=trace)
    LAST_RESULTS = res

    y = np.empty((B, S, D), dtype=np.float32)
    for c in range(8):
        b, t = c // 4, c % 4
        y[b, t * T:(t + 1) * T, :] = res.results[c]["out"]
    return y
